# revision 27
# baseline (speedup 1.0000x reference)
"""Trainium2 Bass kernel for nn_MultiHeadAttn (dense transformer block:
QKV proj -> causal MHA -> out proj -> residual -> LayerNorm).

Sharding: tensor-parallel over the 16 heads across 8 NeuronCores (2 heads
per core). Each core computes Q/K/V projections for its heads over all
tokens (weight-stationary matmuls producing transposed q/k/v; V is then
PE-transposed to token-major), flash-style causal attention (scores kept
transposed [k, q] so the softmax denominator comes from an appended
ones-column in V), then the normalized per-head attention vectors for
BOTH batches are exchanged with a single on-chip AllToAll so that each
core holds all 16 heads for 1/8 of the token rows. Each core then applies
the output projection, residual add and LayerNorm for its token rows. The
host only slices/stacks inputs and concatenates the 8 output chunks.

A single AllToAll (instead of one per batch) matters here: every
collective is a cross-core barrier, and barriers amplify core-speed
variance into wall-clock time.
"""

import os
import sys

import numpy as np

try:
    import concourse.bass as bass  # noqa: F401
except ImportError:  # pragma: no cover
    sys.path.insert(0, "/opt/trn_rl_repo")

import ml_dtypes

import concourse.bass as bass
import concourse.mybir as mybir
import concourse.tile as tile
from concourse import bacc
from concourse.bass_utils import run_bass_kernel_spmd
from concourse.masks import make_identity, make_upper_triangular

# Problem constants
T_FULL = 2048
B = 2
D_MODEL = 1024
N_HEAD = 16
D_HEAD = 64
LN_EPS = 1e-5
N_CORES = 8
SCALE = 1.0 / (D_HEAD**0.5)
EXP_BIAS = -3.0  # scores are in [-3.3, 3.3] for this problem; keeps exp <= ~1.4

P = 128
KCH = D_MODEL // P  # 8 contraction chunks
IB = 512  # i-block (query block) width

F32 = mybir.dt.float32
BF16 = mybir.dt.bfloat16

# Stash of the most recent run's BassKernelResults (for test harnesses).
LAST_RESULT = None


def build_program(t=T_FULL, n_cores=N_CORES, repeat=1, no_collective=False, apply_gb=True,
                  skip_attn=False, skip_proj=False, skip_outproj=False):
    """Builds the SPMD Bass program (same program on every core).

    repeat > 1 re-emits the whole kernel body (everything except constant
    weight loads) that many times — used only for wall-clock timing.
    skip_* flags drop whole phases (output becomes garbage) — used only
    for cost attribution during optimization.
    """
    nh_loc = 2  # heads per core
    n_ib = t // IB  # i-blocks per batch
    nt = t // P  # 128-token tiles per batch
    cs = t // n_cores  # per-batch token chunk per core (A2A shard)
    assert cs % P == 0, "need t >= 128*n_cores for per-batch A2A tiling"
    tiles_pb = cs // P  # 128-row output tiles per batch per core
    n_it = B * tiles_pb  # 128-row output tiles per core

    nc = bacc.Bacc(
        "TRN2", target_bir_lowering=False, debug=False, num_devices=n_cores
    )

    # Kernel I/O (per-core tensors; host supplies per-core contents)
    # wqkv columns: [0:128) = q (scale folded), [128:256) = k, [256:384) = v
    hT_d = nc.dram_tensor("hT", [B, KCH, P, t], BF16, kind="ExternalInput").ap()
    wqkv_d = nc.dram_tensor("wqkv", [KCH, P, 3 * nh_loc * D_HEAD], BF16, kind="ExternalInput").ap()
    wo_d = nc.dram_tensor("wo", [KCH, P, D_MODEL], BF16, kind="ExternalInput").ap()
    hres_d = nc.dram_tensor("hres", [n_it, P, D_MODEL], F32, kind="ExternalInput").ap()
    g_d = nc.dram_tensor("lng", [D_MODEL], F32, kind="ExternalInput").ap()
    b_d = nc.dram_tensor("lnb", [D_MODEL], F32, kind="ExternalInput").ap()
    out_d = nc.dram_tensor("out", [n_it, P, D_MODEL], F32, kind="ExternalOutput").ap()

    def _perm(ap_obj, order):
        """DRAM AP with its iteration dims permuted (strides preserved)."""
        return bass.AP(
            tensor=ap_obj.tensor,
            offset=ap_obj.offset,
            ap=[ap_obj.ap[i] for i in order],
        )

    with tile.TileContext(nc) as tc:
        with (
            tc.tile_pool(name="consts", bufs=1) as consts,
            tc.tile_pool(name="hpool", bufs=1) as hpool,
            tc.tile_pool(name="qkvp", bufs=1) as qkvp,
            tc.tile_pool(name="expp", bufs=6) as expp,
            tc.tile_pool(name="work", bufs=3) as work,
            tc.tile_pool(name="defer", bufs=n_it) as defer_pool,
            tc.tile_pool(name="avsb", bufs=3) as avsb,
            tc.tile_pool(name="pproj", bufs=2, space="PSUM") as pproj,
            tc.tile_pool(name="psc", bufs=2, space="PSUM") as psc,
            tc.tile_pool(name="pav", bufs=2, space="PSUM") as pav,
            tc.tile_pool(name="dram", bufs=1, space="DRAM") as dram,
        ):
            # ---- constants / weights needed for the first phases ----
            # (wo / g / b / hres are only needed after the AllToAll; their
            # DMAs are emitted late so they don't delay the hT load.)
            wqkv_sb = consts.tile([P, KCH, 3 * nh_loc * D_HEAD], BF16)
            # single DMA: DRAM dims (kc, p, c) iterated as (p, kc, c)
            nc.sync.dma_start(out=wqkv_sb, in_=_perm(wqkv_d, (1, 0, 2)))

            eps_sb = consts.tile([P, 1], F32)
            nc.vector.memset(eps_sb, LN_EPS)
            expb_sb = consts.tile([P, 1], F32)
            nc.vector.memset(expb_sb, EXP_BIAS)

            # [128,128] bf16 identity (for PE transposes of vT)
            ident = consts.tile([P, P], BF16)
            make_identity(nc, ident)
            # [128,128] bf16 mask: 1.0 where j <= i (upper triangle incl diag)
            m1 = consts.tile([P, P], BF16)
            make_upper_triangular(nc, m1, val=1.0, diag=True)
            # [128,256] mask for the odd diagonal tile of a pair:
            # cols 0:128 all-zero (fully masked), cols 128:256 triangle
            m2 = consts.tile([P, 2 * P], BF16)
            nc.gpsimd.memset(m2[:, 0:P], 0.0)
            make_upper_triangular(nc, m2[:, P : 2 * P], val=1.0, diag=True)

            wo_sb = consts.tile([P, KCH, D_MODEL], BF16)
            hres_sb = consts.tile([P, n_it, D_MODEL], F32)
            if apply_gb:
                g_sb = consts.tile([P, D_MODEL], F32)
                b_sb = consts.tile([P, D_MODEL], F32)

            for _rep in range(repeat):
                # ---- A2A buffers (a single exchange covering both batches;
                # destination core c receives, for each batch, its token
                # chunk of this core's two heads at columns b*cs..b*cs+cs) ----
                av_in = dram.tile([n_cores, P, B * cs], BF16, name="av_in")
                av_out = dram.tile([n_cores, P, B * cs], BF16, name="av_out")

                # q/k/v kept transposed: [128 rows = 2 heads x 64 dims, tokens]
                qT_sb = qkvp.tile([P, B, t], BF16)
                kT_sb = qkvp.tile([P, B, t], BF16)
                vT_sb = qkvp.tile([P, B, t], BF16)
                # vext[b]: [128, nt, 2, 65]; col 64 of each head stays 1.0
                # (ones-column sumexp trick); filled by PE transposes of vT
                vext = []
                for b in range(B):
                    v = qkvp.tile([P, nt, nh_loc, D_HEAD + 1], BF16, name=f"vext_{b}")
                    if skip_proj:
                        nc.vector.memset(v, 0.01)
                    nc.vector.memset(v[:, :, :, D_HEAD : D_HEAD + 1], 1.0)
                    vext.append(v)

                nsub = D_MODEL // 512  # bn_stats subgroups
                po_parity = 0
                deferred = []

                # Per-batch pipeline: hT DMA -> projections -> attention ->
                # AllToAll -> out-projection/LN. hpool bufs=1 reuses the hT
                # slot across batches (batch 1's DMA overlaps batch 0's
                # attention, which no longer reads hT).
                for b in range(B):
                    # ---- hT load (one DMA per contraction chunk) ----
                    hT_b = hpool.tile([P, KCH, t], BF16, name="hT_b")
                    if not skip_proj:
                        for k in range(KCH):
                            nc.sync.dma_start(out=hT_b[:, k, :], in_=hT_d[b, k])
                    elif b == 0 and _rep == 0:
                        nc.vector.memset(qT_sb, 0.01)
                        nc.vector.memset(kT_sb, 0.01)

                    # ---- Q/K/V projections, all weight-stationary ----
                    for nb in range(0 if skip_proj else t // 512):
                        for mt in range(3):  # 0 -> q, 1 -> k, 2 -> v
                            ps = pproj.tile([P, 512], F32, tag="proj", name="ps_qkv")
                            for k in range(KCH):
                                nc.tensor.matmul(
                                    ps,
                                    lhsT=wqkv_sb[:, k, mt * P : (mt + 1) * P],
                                    rhs=hT_b[:, k, nb * 512 : (nb + 1) * 512],
                                    start=(k == 0),
                                    stop=(k == KCH - 1),
                                )
                            dst = (qT_sb, kT_sb, vT_sb)[mt][:, b, nb * 512 : (nb + 1) * 512]
                            nc.vector.tensor_copy(dst, ps)
                        # transpose this group's vT into token-major vext:
                        # 4 [128,128] PE transposes into one PSUM tile, then
                        # one strided copy into vext (both heads at once)
                        pst = pproj.tile([P, 4, P], BF16, tag="proj", name="pst")
                        for i in range(4):
                            jt = 4 * nb + i
                            nc.tensor.transpose(
                                pst[:, i, :],
                                vT_sb[:, b, jt * P : (jt + 1) * P],
                                ident,
                            )
                        dst = vext[b][:, 4 * nb : 4 * nb + 4, :, 0:D_HEAD]
                        src = bass.AP(
                            tensor=pst.tensor,
                            offset=pst.offset,
                            ap=[pst.ap[0], pst.ap[1], [D_HEAD, nh_loc], [1, D_HEAD]],
                        )
                        nc.vector.tensor_copy(dst, src)

                    # ---- attention ----
                    # scores kept transposed: s[j, i] for j-tile (128 keys) x
                    # i-block (512 queries); softmax over j via the ones
                    # column in V.
                    for ib in range(n_ib):
                        if skip_attn:
                            avt = avsb.tile([D_HEAD, nh_loc, 512], BF16, tag="avt", name="avt")
                            nc.vector.memset(avt, 0.01)
                            for c in range(IB // cs):
                                nc.sync.dma_start(
                                    out=bass.AP(
                                        tensor=av_in.tensor,
                                        offset=av_in.offset
                                        + (ib * (IB // cs) + c) * P * B * cs
                                        + b * cs,
                                        ap=[[B * cs, D_HEAD], [D_HEAD * B * cs, nh_loc], [1, cs]],
                                    ),
                                    in_=bass.AP(
                                        tensor=avt.tensor,
                                        offset=avt.offset + c * cs,
                                        ap=[avt.ap[0], [IB, nh_loc], [1, cs]],
                                    ),
                                )
                            continue
                        njt = 4 * ib + 4  # causal: j-tiles 0..4ib+3
                        avps = [
                            pav.tile([D_HEAD + 1, 512], F32, tag="av", name=f"avps{h}")
                            for h in range(nh_loc)
                        ]
                        njp = njt // 2

                        def pair_off(jp):
                            # causal trim: both tiles of a pair compute query
                            # columns [o0, 512); the odd tile's extra 128
                            # columns are invalid and masked via m2.
                            jt0, jt1 = 2 * jp, 2 * jp + 1
                            o0 = max(0, jt0 * P - ib * IB)
                            o1 = max(0, jt1 * P - ib * IB)
                            return jt0, jt1, o0, o1, jt1 * P - ib * IB >= 0

                        def emit_scores(jp, h):
                            jt0, jt1, o0, _, _ = pair_off(jp)
                            base = h * D_HEAD
                            scp = psc.tile([P, 2, 512], F32, tag="sc", name="scp")
                            for jj, jt in ((0, jt0), (1, jt1)):
                                nc.tensor.matmul(
                                    scp[:, jj, o0:512],
                                    lhsT=kT_sb[base : base + D_HEAD, b, jt * P : (jt + 1) * P],
                                    rhs=qT_sb[base : base + D_HEAD, b, ib * IB + o0 : (ib + 1) * IB],
                                    start=True,
                                    stop=True,
                                )
                            return scp

                        # software pipeline: the next pair's score matmuls are
                        # emitted BEFORE this pair's AV matmuls so the PE feeds
                        # the (pacing) Scalar engine as early as possible
                        scp_cur = [emit_scores(0, h) for h in range(nh_loc)]
                        for jp in range(njp):
                            jt0, jt1, o0, o1, diag = pair_off(jp)
                            expts = []
                            for h in range(nh_loc):
                                expt = expp.tile([P, 2, 512], BF16, tag="exp", name="expt")
                                nc.scalar.activation(
                                    expt[:, :, o0:512],
                                    scp_cur[h][:, :, o0:512],
                                    mybir.ActivationFunctionType.Exp,
                                    bias=expb_sb,
                                )
                                expts.append(expt)
                            if jp + 1 < njp:
                                scp_cur = [emit_scores(jp + 1, h) for h in range(nh_loc)]
                            for h in range(nh_loc):
                                expt = expts[h]
                                if diag:
                                    nc.vector.tensor_mul(
                                        expt[:, 0, o0 : o0 + P], expt[:, 0, o0 : o0 + P], m1
                                    )
                                    nc.vector.tensor_mul(
                                        expt[:, 1, o0 : o0 + 2 * P],
                                        expt[:, 1, o0 : o0 + 2 * P],
                                        m2,
                                    )
                                for jj, jt, oj in ((0, jt0, o0), (1, jt1, o1)):
                                    nc.tensor.matmul(
                                        avps[h][:, oj:512],
                                        lhsT=vext[b][:, jt, h, :],
                                        rhs=expt[:, jj, oj:512],
                                        start=(jt == 0),
                                        stop=(jt == njt - 1),
                                    )
                        # normalize by sumexp (row 64) and ship to the A2A
                        # buffer; both heads batched through one reciprocal /
                        # broadcast / DMA.
                        # sumexp rows: PSUM@p64 -> SBUF@p0 copies (exact),
                        # then reciprocal from SBUF@p0 (approx_fast can't
                        # read shifted PSUM), then broadcast from p0.
                        srow = work.tile([1, nh_loc, 512], F32, tag="srow", name="srow")
                        for h in range(nh_loc):
                            nc.vector.tensor_copy(
                                srow[:, h, :], avps[h][D_HEAD : D_HEAD + 1, :]
                            )
                        rt = work.tile([1, nh_loc, 512], F32, tag="rt", name="rt")
                        nc.vector.reciprocal_approx_fast(out=rt, in_=srow)
                        rb = work.tile([D_HEAD, nh_loc, 512], F32, tag="rb", name="rb")
                        nc.gpsimd.partition_broadcast(rb, rt)
                        avt = avsb.tile([D_HEAD, nh_loc, 512], BF16, tag="avt", name="avt")
                        for h in range(nh_loc):
                            nc.vector.tensor_mul(
                                avt[:, h, :], avps[h][0:D_HEAD, :], rb[:, h, :]
                            )
                        # ship to the A2A buffer, one DMA per token chunk
                        # (both heads per DMA):
                        # av_in[ib*IB//cs + c, h*64 + d, b*cs + o] <- avt[d, h, c*cs + o]
                        nchk = IB // cs
                        assert nchk * cs == IB
                        for c in range(nchk):
                            out_ap = bass.AP(
                                tensor=av_in.tensor,
                                offset=av_in.offset
                                + (ib * nchk + c) * P * B * cs
                                + b * cs,
                                ap=[[B * cs, D_HEAD], [D_HEAD * B * cs, nh_loc], [1, cs]],
                            )
                            in_ap = bass.AP(
                                tensor=avt.tensor,
                                offset=avt.offset + c * cs,
                                ap=[avt.ap[0], [IB, nh_loc], [1, cs]],
                            )
                            nc.sync.dma_start(out=out_ap, in_=in_ap)

                # ---- single AllToAll covering both batches ----
                if no_collective:
                    for k in range(n_cores):
                        nc.sync.dma_start(out=av_out[k], in_=av_in[k])
                else:
                    nc.gpsimd.collective_compute(
                        "AllToAll",
                        mybir.AluOpType.bypass,
                        replica_groups=[list(range(n_cores))],
                        ins=[av_in.opt()],
                        outs=[av_out.opt()],
                    )

                # ---- output projection + residual + LayerNorm for this
                # core's token rows (both batches)
                if _rep == 0:
                    # late-phase constants (emitted here so the DMA queues
                    # serve hT and the qkv weights first at kernel start)
                    nc.sync.dma_start(out=wo_sb, in_=_perm(wo_d, (1, 0, 2)))
                    nc.sync.dma_start(out=hres_sb, in_=_perm(hres_d, (1, 0, 2)))
                    if apply_gb:
                        nc.sync.dma_start(
                            out=g_sb,
                            in_=bass.AP(tensor=g_d.tensor, offset=g_d.offset, ap=[[0, P], *g_d.ap]),
                        )
                        nc.sync.dma_start(
                            out=b_sb,
                            in_=bass.AP(tensor=b_d.tensor, offset=b_d.offset, ap=[[0, P], *b_d.ap]),
                        )

                avg_sb = qkvp.tile([P, n_cores, B * cs], BF16, tag="avg", bufs=1, name="avg_sb")
                nc.sync.dma_start(
                    out=avg_sb,
                    in_=bass.AP(
                        tensor=av_out.tensor,
                        offset=av_out.offset,
                        ap=[av_out.ap[1], av_out.ap[0], av_out.ap[2]],
                    ),
                )

                if skip_outproj and _rep == 0:
                    xz = work.tile([P, D_MODEL], F32, tag="xn", name="xz")
                    nc.vector.memset(xz, 0.0)
                    for it in range(n_it):
                        nc.sync.dma_start(out=out_d[it], in_=xz)
                for it in range(0 if skip_outproj else n_it):
                    bb_, i2 = it // tiles_pb, it % tiles_pb
                    # alternate PSUM pools so tile it+1's matmuls pipeline
                    # with tile it's LayerNorm (attention is fully done here)
                    popool = pproj if po_parity == 0 else pav
                    potag = "proj" if po_parity == 0 else "av"
                    po_parity ^= 1
                    pos = [
                        popool.tile([P, 512], F32, tag=potag, name=f"po{nh}")
                        for nh in range(2)
                    ]
                    for nh in range(2):
                        for k in range(n_cores):
                            nc.tensor.matmul(
                                pos[nh],
                                lhsT=avg_sb[:, k, bb_ * cs + i2 * P : bb_ * cs + (i2 + 1) * P],
                                rhs=wo_sb[:, k, nh * 512 : (nh + 1) * 512],
                                start=(k == 0),
                                stop=(k == n_cores - 1),
                            )
                    x = defer_pool.tile([P, D_MODEL], F32, tag="x", name="x")
                    for nh in range(2):
                        nc.vector.tensor_add(
                            x[:, nh * 512 : (nh + 1) * 512],
                            pos[nh],
                            hres_sb[:, it, nh * 512 : (nh + 1) * 512],
                        )
                    stats = work.tile([P, nsub, 6], F32, tag="stats", name="stats")
                    for s in range(nsub):
                        nc.vector.bn_stats(stats[:, s, :], x[:, s * 512 : (s + 1) * 512])
                    mv = defer_pool.tile([P, 2], F32, tag="mv", name="mv")
                    nc.vector.bn_aggr(mv, stats)
                    # the sqrt + scale are deferred to the kernel tail so
                    # the sqrt ACT-table load doesn't thrash with the
                    # attention exps (different table sets)
                    deferred.append((it, x, mv))

                # ---- deferred LayerNorm tails (one sqrt table switch) ----
                for it, x, mv in deferred:
                    std = work.tile([P, 1], F32, tag="std", name="std")
                    nc.scalar.activation(
                        std, mv[:, 1:2], mybir.ActivationFunctionType.Sqrt, bias=eps_sb
                    )
                    rstd = work.tile([P, 1], F32, tag="rstd", name="rstd")
                    nc.vector.reciprocal(rstd, std)
                    xn = work.tile([P, D_MODEL], F32, tag="xn", name="xn")
                    nc.vector.tensor_scalar(
                        out=xn,
                        in0=x,
                        scalar1=mv[:, 0:1],
                        scalar2=rstd,
                        op0=mybir.AluOpType.subtract,
                        op1=mybir.AluOpType.mult,
                    )
                    if apply_gb:
                        nc.vector.tensor_mul(xn, xn, g_sb)
                        nc.vector.tensor_add(xn, xn, b_sb)
                    nc.sync.dma_start(out=out_d[it], in_=xn)
                deferred.clear()

    nc.compile()
    return nc


def make_in_maps(h, Wq, Wkv, Wo, ln_g, ln_b, t=T_FULL, n_cores=N_CORES):
    """Builds the per-core input maps (host-side sharding/layout prep)."""
    bf = ml_dtypes.bfloat16
    nh_loc = N_HEAD // n_cores
    cs = t // n_cores
    n_it = B * cs // P

    # hT: [B, KCH, P, t] = h transposed per batch, bf16 (shared by all cores)
    hT = np.ascontiguousarray(h.transpose(1, 2, 0)).reshape(B, KCH, P, t).astype(bf)
    # residual in batch-major token order
    h_bmaj = np.ascontiguousarray(h.transpose(1, 0, 2)).reshape(B * t, D_MODEL)
    g = np.ascontiguousarray(ln_g, dtype=np.float32)
    bvec = np.ascontiguousarray(ln_b, dtype=np.float32)
    wo = np.ascontiguousarray(Wo).reshape(KCH, P, D_MODEL).astype(bf)

    in_maps = []
    for c in range(n_cores):
        heads = [c * nh_loc + i for i in range(nh_loc)]
        # Wq columns for my heads, with the 1/sqrt(d) scale folded in
        wq_cols = [Wq[:, hd * D_HEAD : (hd + 1) * D_HEAD] * SCALE for hd in heads]
        # Wkv: head hd occupies cols [hd*128, hd*128+64) = K, [+64, +128) = V
        wk_cols = [Wkv[:, hd * 2 * D_HEAD : hd * 2 * D_HEAD + D_HEAD] for hd in heads]
        wv_cols = [Wkv[:, hd * 2 * D_HEAD + D_HEAD : (hd + 1) * 2 * D_HEAD] for hd in heads]
        wqkv = np.concatenate(wq_cols + wk_cols + wv_cols, axis=1)  # [1024, 384]
        hres = np.concatenate(
            [h_bmaj[b * t + c * cs : b * t + (c + 1) * cs] for b in range(B)]
        ).reshape(n_it, P, D_MODEL)
        in_maps.append(
            {
                "hT": hT,
                "wqkv": np.ascontiguousarray(wqkv.reshape(KCH, P, 3 * nh_loc * D_HEAD)).astype(bf),
                "wo": wo,
                "hres": np.ascontiguousarray(hres, dtype=np.float32),
                "lng": g,
                "lnb": bvec,
            }
        )
    return in_maps


def assemble_output(results, t=T_FULL, n_cores=N_CORES):
    cs = t // n_cores
    chunks = [results[c]["out"].reshape(B, cs, D_MODEL) for c in range(n_cores)]
    # chunks[c][b] = batch-b tokens [c*cs, (c+1)*cs)
    full = np.concatenate(chunks, axis=1)  # [B, t, D]
    return np.ascontiguousarray(full.transpose(1, 0, 2))


def _numpy_fallback(h, attn_mask, Wq, Wkv, Wo, ln_g, ln_b):
    """Exact reference computation (only used if the mask is not causal)."""
    t, b, _ = h.shape
    hf = h.reshape(t * b, D_MODEL)
    q = (hf @ Wq).reshape(t, b, N_HEAD, D_HEAD)
    kv = (hf @ Wkv).reshape(t, b, N_HEAD, 2 * D_HEAD)
    k, v = kv[..., :D_HEAD], kv[..., D_HEAD:]
    s = np.einsum("ibnd,jbnd->ijbn", q, k) * SCALE
    s = np.where(attn_mask[:, :, :, None], -np.inf, s)
    s = s - s.max(axis=1, keepdims=True)
    p = np.exp(s)
    p = p / p.sum(axis=1, keepdims=True)
    av = np.einsum("ijbn,jbnd->ibnd", p, v).reshape(t, b, N_HEAD * D_HEAD)
    ao = av @ Wo
    x = h + ao
    mu = x.mean(axis=-1, keepdims=True)
    var = ((x - mu) ** 2).mean(axis=-1, keepdims=True)
    return ((x - mu) / np.sqrt(var + LN_EPS) * ln_g + ln_b).astype(np.float32)


_PROGRAM_CACHE = {}


def kernel(h, attn_mask, Wq, Wkv, Wo, ln_g, ln_b):
    global LAST_RESULT
    h = np.asarray(h, dtype=np.float32)
    attn_mask = np.asarray(attn_mask)
    Wq = np.asarray(Wq, dtype=np.float32)
    Wkv = np.asarray(Wkv, dtype=np.float32)
    Wo = np.asarray(Wo, dtype=np.float32)
    ln_g = np.asarray(ln_g, dtype=np.float32)
    ln_b = np.asarray(ln_b, dtype=np.float32)

    t = h.shape[0]
    causal = np.triu(np.ones((t, t), dtype=bool), k=1)
    if not np.array_equal(attn_mask, np.broadcast_to(causal[:, :, None], attn_mask.shape)):
        return _numpy_fallback(h, attn_mask, Wq, Wkv, Wo, ln_g, ln_b)

    apply_gb = not (np.all(ln_g == 1.0) and np.all(ln_b == 0.0))
    key = (t, apply_gb)
    if key not in _PROGRAM_CACHE:
        _PROGRAM_CACHE[key] = build_program(t=t, apply_gb=apply_gb)
    nc = _PROGRAM_CACHE[key]

    in_maps = make_in_maps(h, Wq, Wkv, Wo, ln_g, ln_b, t=t)
    res = run_bass_kernel_spmd(
        nc,
        in_maps,
        core_ids=list(range(N_CORES)),
        trace=bool(int(os.environ.get("KERNEL_TRACE", "0"))),
    )
    LAST_RESULT = res
    return assemble_output(res.results, t=t)


if __name__ == "__main__":
    # quick smoke: random small check vs numpy fallback path is not possible
    # (device required); just build the program.
    build_program()
    print("program built ok")



# revision 32
# speedup vs baseline: 1.0325x; 1.0325x over previous
"""Trainium2 Bass kernel for nn_MultiHeadAttn (dense transformer block:
QKV proj -> causal MHA -> out proj -> residual -> LayerNorm).

Sharding: tensor-parallel over the 16 heads across 8 NeuronCores (2 heads
per core). Each core computes Q/K/V projections for its heads over all
tokens (weight-stationary matmuls producing transposed q/k/v; V is then
PE-transposed to token-major), flash-style causal attention (scores kept
transposed [k, q] so the softmax denominator comes from an appended
ones-column in V), then the normalized per-head attention vectors for
BOTH batches are exchanged with a single on-chip AllToAll so that each
core holds all 16 heads for 1/8 of the token rows. Each core then applies
the output projection, residual add and LayerNorm for its token rows. The
host only slices/stacks inputs and concatenates the 8 output chunks.

A single AllToAll (instead of one per batch) matters here: every
collective is a cross-core barrier, and barriers amplify core-speed
variance into wall-clock time.
"""

import os
import sys

import numpy as np

try:
    import concourse.bass as bass  # noqa: F401
except ImportError:  # pragma: no cover
    sys.path.insert(0, "/opt/trn_rl_repo")

import ml_dtypes

import concourse.bass as bass
import concourse.mybir as mybir
import concourse.tile as tile
from concourse import bacc
from concourse.bass_utils import run_bass_kernel_spmd
from concourse.masks import make_identity, make_upper_triangular

# Problem constants
T_FULL = 2048
B = 2
D_MODEL = 1024
N_HEAD = 16
D_HEAD = 64
LN_EPS = 1e-5
N_CORES = 8
SCALE = 1.0 / (D_HEAD**0.5)
EXP_BIAS = -3.0  # scores are in [-3.3, 3.3] for this problem; keeps exp <= ~1.4

P = 128
KCH = D_MODEL // P  # 8 contraction chunks
IB = 512  # i-block (query block) width

F32 = mybir.dt.float32
BF16 = mybir.dt.bfloat16

# Stash of the most recent run's BassKernelResults (for test harnesses).
LAST_RESULT = None


def build_program(t=T_FULL, n_cores=N_CORES, repeat=1, no_collective=False, apply_gb=True,
                  skip_attn=False, skip_proj=False, skip_outproj=False):
    """Builds the SPMD Bass program (same program on every core).

    repeat > 1 re-emits the whole kernel body (everything except constant
    weight loads) that many times — used only for wall-clock timing.
    skip_* flags drop whole phases (output becomes garbage) — used only
    for cost attribution during optimization.
    """
    nh_loc = 2  # heads per core
    n_ib = t // IB  # i-blocks per batch
    nt = t // P  # 128-token tiles per batch
    cs = t // n_cores  # per-batch token chunk per core (A2A shard)
    assert cs % P == 0, "need t >= 128*n_cores for per-batch A2A tiling"
    tiles_pb = cs // P  # 128-row output tiles per batch per core
    n_it = B * tiles_pb  # 128-row output tiles per core

    nc = bacc.Bacc(
        "TRN2", target_bir_lowering=False, debug=False, num_devices=n_cores
    )

    # Kernel I/O (per-core tensors; host supplies per-core contents)
    # wqkv columns: [0:128) = q (scale folded), [128:256) = k, [256:384) = v
    hT_d = nc.dram_tensor("hT", [B, KCH, P, t], BF16, kind="ExternalInput").ap()
    wqkv_d = nc.dram_tensor("wqkv", [KCH, P, 3 * nh_loc * D_HEAD], BF16, kind="ExternalInput").ap()
    wo_d = nc.dram_tensor("wo", [KCH, P, D_MODEL], BF16, kind="ExternalInput").ap()
    hres_d = nc.dram_tensor("hres", [n_it, P, D_MODEL], F32, kind="ExternalInput").ap()
    g_d = nc.dram_tensor("lng", [D_MODEL], F32, kind="ExternalInput").ap()
    b_d = nc.dram_tensor("lnb", [D_MODEL], F32, kind="ExternalInput").ap()
    out_d = nc.dram_tensor("out", [n_it, P, D_MODEL], F32, kind="ExternalOutput").ap()

    def _perm(ap_obj, order):
        """DRAM AP with its iteration dims permuted (strides preserved)."""
        return bass.AP(
            tensor=ap_obj.tensor,
            offset=ap_obj.offset,
            ap=[ap_obj.ap[i] for i in order],
        )

    with tile.TileContext(nc) as tc:
        with (
            tc.tile_pool(name="consts", bufs=1) as consts,
            tc.tile_pool(name="hpool", bufs=1) as hpool,
            tc.tile_pool(name="qkvp", bufs=1) as qkvp,
            tc.tile_pool(name="expp", bufs=6) as expp,
            tc.tile_pool(name="work", bufs=3) as work,
            tc.tile_pool(name="defer", bufs=n_it) as defer_pool,
            tc.tile_pool(name="avsb", bufs=3) as avsb,
            tc.tile_pool(name="pproj", bufs=2, space="PSUM") as pproj,
            tc.tile_pool(name="psc", bufs=2, space="PSUM") as psc,
            tc.tile_pool(name="pav", bufs=2, space="PSUM") as pav,
            tc.tile_pool(name="dram", bufs=1, space="DRAM") as dram,
        ):
            # ---- constants / weights needed for the first phases ----
            # (wo / g / b / hres are only needed after the AllToAll; their
            # DMAs are emitted late so they don't delay the hT load.)
            wqkv_sb = consts.tile([P, KCH, 3 * nh_loc * D_HEAD], BF16)
            # single DMA: DRAM dims (kc, p, c) iterated as (p, kc, c)
            nc.sync.dma_start(out=wqkv_sb, in_=_perm(wqkv_d, (1, 0, 2)))

            eps_sb = consts.tile([P, 1], F32)
            nc.vector.memset(eps_sb, LN_EPS)
            expb_sb = consts.tile([P, 1], F32)
            nc.vector.memset(expb_sb, EXP_BIAS)

            # [128,128] bf16 identity (for PE transposes of vT)
            ident = consts.tile([P, P], BF16)
            make_identity(nc, ident)
            # [128,128] bf16 mask: 1.0 where j <= i (upper triangle incl diag)
            m1 = consts.tile([P, P], BF16)
            make_upper_triangular(nc, m1, val=1.0, diag=True)
            # [128,256] mask for the odd diagonal tile of a pair:
            # cols 0:128 all-zero (fully masked), cols 128:256 triangle
            m2 = consts.tile([P, 2 * P], BF16)
            nc.gpsimd.memset(m2[:, 0:P], 0.0)
            make_upper_triangular(nc, m2[:, P : 2 * P], val=1.0, diag=True)

            wo_sb = consts.tile([P, KCH, D_MODEL], BF16)
            hres_sb = consts.tile([P, n_it, D_MODEL], F32)
            if apply_gb:
                g_sb = consts.tile([P, D_MODEL], F32)
                b_sb = consts.tile([P, D_MODEL], F32)

            for _rep in range(repeat):
                # ---- A2A buffers (a single exchange covering both batches;
                # destination core c receives, for each batch, its token
                # chunk of this core's two heads at columns b*cs..b*cs+cs) ----
                av_in = dram.tile([n_cores, P, B * cs], BF16, name="av_in")
                av_out = dram.tile([n_cores, P, B * cs], BF16, name="av_out")

                # q/k/v kept transposed: [128 rows = 2 heads x 64 dims, tokens]
                qT_sb = qkvp.tile([P, B, t], BF16)
                kT_sb = qkvp.tile([P, B, t], BF16)
                vT_sb = qkvp.tile([P, B, t], BF16)
                # vext[b]: [128, nt, 2, 65]; col 64 of each head stays 1.0
                # (ones-column sumexp trick); filled by PE transposes of vT
                vext = []
                for b in range(B):
                    v = qkvp.tile([P, nt, nh_loc, D_HEAD + 1], BF16, name=f"vext_{b}")
                    if skip_proj:
                        nc.vector.memset(v, 0.01)
                    nc.vector.memset(v[:, :, :, D_HEAD : D_HEAD + 1], 1.0)
                    vext.append(v)

                nsub = D_MODEL // 512  # bn_stats subgroups
                po_parity = 0
                deferred = []

                # Pipeline: proj(b0) first; then attention(b) with batch
                # b+1's projection groups interleaved after each i-block.
                # Attention is Act-bound (exps) while projections are
                # PE-bound, so the PE fills its idle slack with projection
                # matmuls and the serial projection phase between batches
                # disappears. PSUM is conflict-free: projections use the
                # pproj ring, attention uses psc/pav.
                def load_hT(b):
                    # hpool bufs=1: this DMA waits (WAR) until the previous
                    # batch's projections finish reading the slot
                    hT_b = hpool.tile([P, KCH, t], BF16, name="hT_b")
                    for k in range(KCH):
                        nc.sync.dma_start(out=hT_b[:, k, :], in_=hT_d[b, k])
                    return hT_b

                def emit_proj_group(b, hT_b, nb):
                    # ---- Q/K/V projections, all weight-stationary ----
                    for mt in range(3):  # 0 -> q, 1 -> k, 2 -> v
                        ps = pproj.tile([P, 512], F32, tag="proj", name="ps_qkv")
                        for k in range(KCH):
                            nc.tensor.matmul(
                                ps,
                                lhsT=wqkv_sb[:, k, mt * P : (mt + 1) * P],
                                rhs=hT_b[:, k, nb * 512 : (nb + 1) * 512],
                                start=(k == 0),
                                stop=(k == KCH - 1),
                            )
                        dst = (qT_sb, kT_sb, vT_sb)[mt][:, b, nb * 512 : (nb + 1) * 512]
                        nc.vector.tensor_copy(dst, ps)
                    # transpose this group's vT into token-major vext:
                    # 4 [128,128] PE transposes into one PSUM tile, then
                    # one strided copy into vext (both heads at once)
                    pst = pproj.tile([P, 4, P], BF16, tag="proj", name="pst")
                    for i in range(4):
                        jt = 4 * nb + i
                        nc.tensor.transpose(
                            pst[:, i, :],
                            vT_sb[:, b, jt * P : (jt + 1) * P],
                            ident,
                        )
                    dst = vext[b][:, 4 * nb : 4 * nb + 4, :, 0:D_HEAD]
                    src = bass.AP(
                        tensor=pst.tensor,
                        offset=pst.offset,
                        ap=[pst.ap[0], pst.ap[1], [D_HEAD, nh_loc], [1, D_HEAD]],
                    )
                    nc.vector.tensor_copy(dst, src)

                def emit_attn_ib(b, ib):
                    # ---- attention ----
                    # scores kept transposed: s[j, i] for j-tile (128 keys)
                    # x i-block (512 queries); softmax over j via the ones
                    # column in V.
                    if True:
                        if skip_attn:
                            avt = avsb.tile([D_HEAD, nh_loc, 512], BF16, tag="avt", name="avt")
                            nc.vector.memset(avt, 0.01)
                            for c in range(IB // cs):
                                nc.sync.dma_start(
                                    out=bass.AP(
                                        tensor=av_in.tensor,
                                        offset=av_in.offset
                                        + (ib * (IB // cs) + c) * P * B * cs
                                        + b * cs,
                                        ap=[[B * cs, D_HEAD], [D_HEAD * B * cs, nh_loc], [1, cs]],
                                    ),
                                    in_=bass.AP(
                                        tensor=avt.tensor,
                                        offset=avt.offset + c * cs,
                                        ap=[avt.ap[0], [IB, nh_loc], [1, cs]],
                                    ),
                                )
                            return
                        njt = 4 * ib + 4  # causal: j-tiles 0..4ib+3
                        avps = [
                            pav.tile([D_HEAD + 1, 512], F32, tag="av", name=f"avps{h}")
                            for h in range(nh_loc)
                        ]
                        njp = njt // 2

                        def pair_off(jp):
                            # causal trim: both tiles of a pair compute query
                            # columns [o0, 512); the odd tile's extra 128
                            # columns are invalid and masked via m2.
                            jt0, jt1 = 2 * jp, 2 * jp + 1
                            o0 = max(0, jt0 * P - ib * IB)
                            o1 = max(0, jt1 * P - ib * IB)
                            return jt0, jt1, o0, o1, jt1 * P - ib * IB >= 0

                        def emit_scores(jp, h):
                            jt0, jt1, o0, _, _ = pair_off(jp)
                            base = h * D_HEAD
                            scp = psc.tile([P, 2, 512], F32, tag="sc", name="scp")
                            for jj, jt in ((0, jt0), (1, jt1)):
                                nc.tensor.matmul(
                                    scp[:, jj, o0:512],
                                    lhsT=kT_sb[base : base + D_HEAD, b, jt * P : (jt + 1) * P],
                                    rhs=qT_sb[base : base + D_HEAD, b, ib * IB + o0 : (ib + 1) * IB],
                                    start=True,
                                    stop=True,
                                )
                            return scp

                        # software pipeline: the next pair's score matmuls are
                        # emitted BEFORE this pair's AV matmuls so the PE feeds
                        # the (pacing) Scalar engine as early as possible
                        scp_cur = [emit_scores(0, h) for h in range(nh_loc)]
                        for jp in range(njp):
                            jt0, jt1, o0, o1, diag = pair_off(jp)
                            expts = []
                            for h in range(nh_loc):
                                expt = expp.tile([P, 2, 512], BF16, tag="exp", name="expt")
                                nc.scalar.activation(
                                    expt[:, :, o0:512],
                                    scp_cur[h][:, :, o0:512],
                                    mybir.ActivationFunctionType.Exp,
                                    bias=expb_sb,
                                )
                                expts.append(expt)
                            if jp + 1 < njp:
                                scp_cur = [emit_scores(jp + 1, h) for h in range(nh_loc)]
                            for h in range(nh_loc):
                                expt = expts[h]
                                if diag:
                                    nc.vector.tensor_mul(
                                        expt[:, 0, o0 : o0 + P], expt[:, 0, o0 : o0 + P], m1
                                    )
                                    nc.vector.tensor_mul(
                                        expt[:, 1, o0 : o0 + 2 * P],
                                        expt[:, 1, o0 : o0 + 2 * P],
                                        m2,
                                    )
                                for jj, jt, oj in ((0, jt0, o0), (1, jt1, o1)):
                                    nc.tensor.matmul(
                                        avps[h][:, oj:512],
                                        lhsT=vext[b][:, jt, h, :],
                                        rhs=expt[:, jj, oj:512],
                                        start=(jt == 0),
                                        stop=(jt == njt - 1),
                                    )
                        # normalize by sumexp (row 64) and ship to the A2A
                        # buffer; both heads batched through one reciprocal /
                        # broadcast / DMA.
                        # sumexp rows: PSUM@p64 -> SBUF@p0 copies (exact),
                        # then reciprocal from SBUF@p0 (approx_fast can't
                        # read shifted PSUM), then broadcast from p0.
                        srow = work.tile([1, nh_loc, 512], F32, tag="srow", name="srow")
                        for h in range(nh_loc):
                            nc.vector.tensor_copy(
                                srow[:, h, :], avps[h][D_HEAD : D_HEAD + 1, :]
                            )
                        rt = work.tile([1, nh_loc, 512], F32, tag="rt", name="rt")
                        nc.vector.reciprocal_approx_fast(out=rt, in_=srow)
                        rb = work.tile([D_HEAD, nh_loc, 512], F32, tag="rb", name="rb")
                        nc.gpsimd.partition_broadcast(rb, rt)
                        avt = avsb.tile([D_HEAD, nh_loc, 512], BF16, tag="avt", name="avt")
                        for h in range(nh_loc):
                            nc.vector.tensor_mul(
                                avt[:, h, :], avps[h][0:D_HEAD, :], rb[:, h, :]
                            )
                        # ship to the A2A buffer, one DMA per token chunk
                        # (both heads per DMA):
                        # av_in[ib*IB//cs + c, h*64 + d, b*cs + o] <- avt[d, h, c*cs + o]
                        nchk = IB // cs
                        assert nchk * cs == IB
                        for c in range(nchk):
                            out_ap = bass.AP(
                                tensor=av_in.tensor,
                                offset=av_in.offset
                                + (ib * nchk + c) * P * B * cs
                                + b * cs,
                                ap=[[B * cs, D_HEAD], [D_HEAD * B * cs, nh_loc], [1, cs]],
                            )
                            in_ap = bass.AP(
                                tensor=avt.tensor,
                                offset=avt.offset + c * cs,
                                ap=[avt.ap[0], [IB, nh_loc], [1, cs]],
                            )
                            nc.sync.dma_start(out=out_ap, in_=in_ap)

                # ---- driver: per-batch pipeline (hT DMA -> projections
                # -> attention); batch b+1's hT DMA overlaps batch b's
                # attention via the hpool WAR ----
                if skip_proj and _rep == 0:
                    nc.vector.memset(qT_sb, 0.01)
                    nc.vector.memset(kT_sb, 0.01)
                for b in range(B):
                    if not skip_proj:
                        hT_b = load_hT(b)
                        for nb in range(t // 512):
                            emit_proj_group(b, hT_b, nb)
                    for ib in range(n_ib):
                        emit_attn_ib(b, ib)

                # ---- single AllToAll covering both batches ----
                if no_collective:
                    for k in range(n_cores):
                        nc.sync.dma_start(out=av_out[k], in_=av_in[k])
                else:
                    nc.gpsimd.collective_compute(
                        "AllToAll",
                        mybir.AluOpType.bypass,
                        replica_groups=[list(range(n_cores))],
                        ins=[av_in.opt()],
                        outs=[av_out.opt()],
                    )

                # ---- output projection + residual + LayerNorm for this
                # core's token rows (both batches)
                if _rep == 0:
                    # late-phase constants (emitted here so the DMA queues
                    # serve hT and the qkv weights first at kernel start)
                    nc.sync.dma_start(out=wo_sb, in_=_perm(wo_d, (1, 0, 2)))
                    nc.sync.dma_start(out=hres_sb, in_=_perm(hres_d, (1, 0, 2)))
                    if apply_gb:
                        nc.sync.dma_start(
                            out=g_sb,
                            in_=bass.AP(tensor=g_d.tensor, offset=g_d.offset, ap=[[0, P], *g_d.ap]),
                        )
                        nc.sync.dma_start(
                            out=b_sb,
                            in_=bass.AP(tensor=b_d.tensor, offset=b_d.offset, ap=[[0, P], *b_d.ap]),
                        )

                avg_sb = qkvp.tile([P, n_cores, B * cs], BF16, tag="avg", bufs=1, name="avg_sb")
                nc.sync.dma_start(
                    out=avg_sb,
                    in_=bass.AP(
                        tensor=av_out.tensor,
                        offset=av_out.offset,
                        ap=[av_out.ap[1], av_out.ap[0], av_out.ap[2]],
                    ),
                )

                if skip_outproj and _rep == 0:
                    xz = work.tile([P, D_MODEL], F32, tag="xn", name="xz")
                    nc.vector.memset(xz, 0.0)
                    for it in range(n_it):
                        nc.sync.dma_start(out=out_d[it], in_=xz)
                for it in range(0 if skip_outproj else n_it):
                    bb_, i2 = it // tiles_pb, it % tiles_pb
                    # alternate PSUM pools so tile it+1's matmuls pipeline
                    # with tile it's LayerNorm (attention is fully done here)
                    popool = pproj if po_parity == 0 else pav
                    potag = "proj" if po_parity == 0 else "av"
                    po_parity ^= 1
                    pos = [
                        popool.tile([P, 512], F32, tag=potag, name=f"po{nh}")
                        for nh in range(2)
                    ]
                    for nh in range(2):
                        for k in range(n_cores):
                            nc.tensor.matmul(
                                pos[nh],
                                lhsT=avg_sb[:, k, bb_ * cs + i2 * P : bb_ * cs + (i2 + 1) * P],
                                rhs=wo_sb[:, k, nh * 512 : (nh + 1) * 512],
                                start=(k == 0),
                                stop=(k == n_cores - 1),
                            )
                    x = defer_pool.tile([P, D_MODEL], F32, tag="x", name="x")
                    for nh in range(2):
                        nc.vector.tensor_add(
                            x[:, nh * 512 : (nh + 1) * 512],
                            pos[nh],
                            hres_sb[:, it, nh * 512 : (nh + 1) * 512],
                        )
                    stats = work.tile([P, nsub, 6], F32, tag="stats", name="stats")
                    for s in range(nsub):
                        nc.vector.bn_stats(stats[:, s, :], x[:, s * 512 : (s + 1) * 512])
                    mv = defer_pool.tile([P, 2], F32, tag="mv", name="mv")
                    nc.vector.bn_aggr(mv, stats)
                    # the sqrt + scale are deferred to the kernel tail so
                    # the sqrt ACT-table load doesn't thrash with the
                    # attention exps (different table sets)
                    deferred.append((it, x, mv))

                # ---- deferred LayerNorm tails (one sqrt table switch) ----
                for it, x, mv in deferred:
                    std = work.tile([P, 1], F32, tag="std", name="std")
                    nc.scalar.activation(
                        std, mv[:, 1:2], mybir.ActivationFunctionType.Sqrt, bias=eps_sb
                    )
                    rstd = work.tile([P, 1], F32, tag="rstd", name="rstd")
                    nc.vector.reciprocal(rstd, std)
                    xn = work.tile([P, D_MODEL], F32, tag="xn", name="xn")
                    nc.vector.tensor_scalar(
                        out=xn,
                        in0=x,
                        scalar1=mv[:, 0:1],
                        scalar2=rstd,
                        op0=mybir.AluOpType.subtract,
                        op1=mybir.AluOpType.mult,
                    )
                    if apply_gb:
                        nc.vector.tensor_mul(xn, xn, g_sb)
                        nc.vector.tensor_add(xn, xn, b_sb)
                    nc.sync.dma_start(out=out_d[it], in_=xn)
                deferred.clear()

    nc.compile()
    return nc


def make_in_maps(h, Wq, Wkv, Wo, ln_g, ln_b, t=T_FULL, n_cores=N_CORES):
    """Builds the per-core input maps (host-side sharding/layout prep)."""
    bf = ml_dtypes.bfloat16
    nh_loc = N_HEAD // n_cores
    cs = t // n_cores
    n_it = B * cs // P

    # hT: [B, KCH, P, t] = h transposed per batch, bf16 (shared by all cores)
    hT = np.ascontiguousarray(h.transpose(1, 2, 0)).reshape(B, KCH, P, t).astype(bf)
    # residual in batch-major token order
    h_bmaj = np.ascontiguousarray(h.transpose(1, 0, 2)).reshape(B * t, D_MODEL)
    g = np.ascontiguousarray(ln_g, dtype=np.float32)
    bvec = np.ascontiguousarray(ln_b, dtype=np.float32)
    wo = np.ascontiguousarray(Wo).reshape(KCH, P, D_MODEL).astype(bf)

    in_maps = []
    for c in range(n_cores):
        heads = [c * nh_loc + i for i in range(nh_loc)]
        # Wq columns for my heads, with the 1/sqrt(d) scale folded in
        wq_cols = [Wq[:, hd * D_HEAD : (hd + 1) * D_HEAD] * SCALE for hd in heads]
        # Wkv: head hd occupies cols [hd*128, hd*128+64) = K, [+64, +128) = V
        wk_cols = [Wkv[:, hd * 2 * D_HEAD : hd * 2 * D_HEAD + D_HEAD] for hd in heads]
        wv_cols = [Wkv[:, hd * 2 * D_HEAD + D_HEAD : (hd + 1) * 2 * D_HEAD] for hd in heads]
        wqkv = np.concatenate(wq_cols + wk_cols + wv_cols, axis=1)  # [1024, 384]
        hres = np.concatenate(
            [h_bmaj[b * t + c * cs : b * t + (c + 1) * cs] for b in range(B)]
        ).reshape(n_it, P, D_MODEL)
        in_maps.append(
            {
                "hT": hT,
                "wqkv": np.ascontiguousarray(wqkv.reshape(KCH, P, 3 * nh_loc * D_HEAD)).astype(bf),
                "wo": wo,
                "hres": np.ascontiguousarray(hres, dtype=np.float32),
                "lng": g,
                "lnb": bvec,
            }
        )
    return in_maps


def assemble_output(results, t=T_FULL, n_cores=N_CORES):
    cs = t // n_cores
    chunks = [results[c]["out"].reshape(B, cs, D_MODEL) for c in range(n_cores)]
    # chunks[c][b] = batch-b tokens [c*cs, (c+1)*cs)
    full = np.concatenate(chunks, axis=1)  # [B, t, D]
    return np.ascontiguousarray(full.transpose(1, 0, 2))


def _numpy_fallback(h, attn_mask, Wq, Wkv, Wo, ln_g, ln_b):
    """Exact reference computation (only used if the mask is not causal)."""
    t, b, _ = h.shape
    hf = h.reshape(t * b, D_MODEL)
    q = (hf @ Wq).reshape(t, b, N_HEAD, D_HEAD)
    kv = (hf @ Wkv).reshape(t, b, N_HEAD, 2 * D_HEAD)
    k, v = kv[..., :D_HEAD], kv[..., D_HEAD:]
    s = np.einsum("ibnd,jbnd->ijbn", q, k) * SCALE
    s = np.where(attn_mask[:, :, :, None], -np.inf, s)
    s = s - s.max(axis=1, keepdims=True)
    p = np.exp(s)
    p = p / p.sum(axis=1, keepdims=True)
    av = np.einsum("ijbn,jbnd->ibnd", p, v).reshape(t, b, N_HEAD * D_HEAD)
    ao = av @ Wo
    x = h + ao
    mu = x.mean(axis=-1, keepdims=True)
    var = ((x - mu) ** 2).mean(axis=-1, keepdims=True)
    return ((x - mu) / np.sqrt(var + LN_EPS) * ln_g + ln_b).astype(np.float32)


_PROGRAM_CACHE = {}


def kernel(h, attn_mask, Wq, Wkv, Wo, ln_g, ln_b):
    global LAST_RESULT
    h = np.asarray(h, dtype=np.float32)
    attn_mask = np.asarray(attn_mask)
    Wq = np.asarray(Wq, dtype=np.float32)
    Wkv = np.asarray(Wkv, dtype=np.float32)
    Wo = np.asarray(Wo, dtype=np.float32)
    ln_g = np.asarray(ln_g, dtype=np.float32)
    ln_b = np.asarray(ln_b, dtype=np.float32)

    t = h.shape[0]
    causal = np.triu(np.ones((t, t), dtype=bool), k=1)
    if not np.array_equal(attn_mask, np.broadcast_to(causal[:, :, None], attn_mask.shape)):
        return _numpy_fallback(h, attn_mask, Wq, Wkv, Wo, ln_g, ln_b)

    apply_gb = not (np.all(ln_g == 1.0) and np.all(ln_b == 0.0))
    key = (t, apply_gb)
    if key not in _PROGRAM_CACHE:
        _PROGRAM_CACHE[key] = build_program(t=t, apply_gb=apply_gb)
    nc = _PROGRAM_CACHE[key]

    in_maps = make_in_maps(h, Wq, Wkv, Wo, ln_g, ln_b, t=t)
    res = run_bass_kernel_spmd(
        nc,
        in_maps,
        core_ids=list(range(N_CORES)),
        trace=bool(int(os.environ.get("KERNEL_TRACE", "0"))),
    )
    LAST_RESULT = res
    return assemble_output(res.results, t=t)


if __name__ == "__main__":
    # quick smoke: random small check vs numpy fallback path is not possible
    # (device required); just build the program.
    build_program()
    print("program built ok")



# revision 33
# speedup vs baseline: 1.2028x; 1.1649x over previous
"""Trainium2 Bass kernel for nn_MultiHeadAttn (dense transformer block:
QKV proj -> causal MHA -> out proj -> residual -> LayerNorm).

Sharding: tensor-parallel over the 16 heads across 8 NeuronCores (2 heads
per core). Each core computes Q/K/V projections for its heads over all
tokens (weight-stationary matmuls producing transposed q/k/v; V is then
PE-transposed to token-major), flash-style causal attention (scores kept
transposed [k, q] so the softmax denominator comes from an appended
ones-column in V), then the normalized per-head attention vectors for
BOTH batches are exchanged with a single on-chip AllToAll so that each
core holds all 16 heads for 1/8 of the token rows. Each core then applies
the output projection, residual add and LayerNorm for its token rows. The
host only slices/stacks inputs and concatenates the 8 output chunks.

A single AllToAll (instead of one per batch) matters here: every
collective is a cross-core barrier, and barriers amplify core-speed
variance into wall-clock time.
"""

import os
import sys

import numpy as np

try:
    import concourse.bass as bass  # noqa: F401
except ImportError:  # pragma: no cover
    sys.path.insert(0, "/opt/trn_rl_repo")

import ml_dtypes

import concourse.bass as bass
import concourse.mybir as mybir
import concourse.tile as tile
from concourse import bacc
from concourse.bass_utils import run_bass_kernel_spmd
from concourse.masks import make_identity, make_upper_triangular

# Problem constants
T_FULL = 2048
B = 2
D_MODEL = 1024
N_HEAD = 16
D_HEAD = 64
LN_EPS = 1e-5
N_CORES = 8
SCALE = 1.0 / (D_HEAD**0.5)
EXP_BIAS = -3.0  # scores are in [-3.3, 3.3] for this problem; keeps exp <= ~1.4

P = 128
KCH = D_MODEL // P  # 8 contraction chunks
IB = 512  # i-block (query block) width

F32 = mybir.dt.float32
BF16 = mybir.dt.bfloat16
F8 = mybir.dt.float8e4

# Stash of the most recent run's BassKernelResults (for test harnesses).
LAST_RESULT = None


def build_program(t=T_FULL, n_cores=N_CORES, repeat=1, no_collective=False, apply_gb=True,
                  skip_attn=False, skip_proj=False, skip_outproj=False):
    """Builds the SPMD Bass program (same program on every core).

    repeat > 1 re-emits the whole kernel body (everything except constant
    weight loads) that many times — used only for wall-clock timing.
    skip_* flags drop whole phases (output becomes garbage) — used only
    for cost attribution during optimization.
    """
    nh_loc = 2  # heads per core
    n_ib = t // IB  # i-blocks per batch
    nt = t // P  # 128-token tiles per batch
    cs = t // n_cores  # per-batch token chunk per core (A2A shard)
    assert cs % P == 0, "need t >= 128*n_cores for per-batch A2A tiling"
    tiles_pb = cs // P  # 128-row output tiles per batch per core
    n_it = B * tiles_pb  # 128-row output tiles per core

    nc = bacc.Bacc(
        "TRN2", target_bir_lowering=False, debug=False, num_devices=n_cores
    )

    # Kernel I/O (per-core tensors; host supplies per-core contents)
    # wqkv columns: [0:128) = q (scale folded), [128:256) = k, [256:384) = v
    hT_d = nc.dram_tensor("hT", [B, KCH, P, t], BF16, kind="ExternalInput").ap()
    wqkv_d = nc.dram_tensor("wqkv", [KCH, P, 3 * nh_loc * D_HEAD], BF16, kind="ExternalInput").ap()
    wo_d = nc.dram_tensor("wo", [KCH, P, D_MODEL], BF16, kind="ExternalInput").ap()
    hres_d = nc.dram_tensor("hres", [n_it, P, D_MODEL], F32, kind="ExternalInput").ap()
    g_d = nc.dram_tensor("lng", [D_MODEL], F32, kind="ExternalInput").ap()
    b_d = nc.dram_tensor("lnb", [D_MODEL], F32, kind="ExternalInput").ap()
    out_d = nc.dram_tensor("out", [n_it, P, D_MODEL], F32, kind="ExternalOutput").ap()

    def _perm(ap_obj, order):
        """DRAM AP with its iteration dims permuted (strides preserved)."""
        return bass.AP(
            tensor=ap_obj.tensor,
            offset=ap_obj.offset,
            ap=[ap_obj.ap[i] for i in order],
        )

    with tile.TileContext(nc) as tc:
        with (
            tc.tile_pool(name="consts", bufs=1) as consts,
            tc.tile_pool(name="hpool", bufs=1) as hpool,
            tc.tile_pool(name="qkvp", bufs=1) as qkvp,
            tc.tile_pool(name="expp", bufs=6) as expp,
            tc.tile_pool(name="work", bufs=3) as work,
            tc.tile_pool(name="defer", bufs=n_it) as defer_pool,
            tc.tile_pool(name="avsb", bufs=3) as avsb,
            tc.tile_pool(name="pproj", bufs=2, space="PSUM") as pproj,
            tc.tile_pool(name="psc", bufs=2, space="PSUM") as psc,
            tc.tile_pool(name="pav", bufs=2, space="PSUM") as pav,
            tc.tile_pool(name="dram", bufs=1, space="DRAM") as dram,
        ):
            # ---- constants / weights needed for the first phases ----
            # (wo / g / b / hres are only needed after the AllToAll; their
            # DMAs are emitted late so they don't delay the hT load.)
            wqkv_sb = consts.tile([P, KCH, 3 * nh_loc * D_HEAD], BF16)
            # single DMA: DRAM dims (kc, p, c) iterated as (p, kc, c)
            nc.sync.dma_start(out=wqkv_sb, in_=_perm(wqkv_d, (1, 0, 2)))

            eps_sb = consts.tile([P, 1], F32)
            nc.vector.memset(eps_sb, LN_EPS)
            expb_sb = consts.tile([P, 1], F32)
            nc.vector.memset(expb_sb, EXP_BIAS)

            # [128,128] bf16 identity (for PE transposes of vT)
            ident = consts.tile([P, P], BF16)
            make_identity(nc, ident)
            # [128,128] bf16 mask: 1.0 where j <= i (upper triangle incl diag)
            m1 = consts.tile([P, P], BF16)
            make_upper_triangular(nc, m1, val=1.0, diag=True)
            # [128,256] mask for the odd diagonal tile of a pair:
            # cols 0:128 all-zero (fully masked), cols 128:256 triangle
            m2 = consts.tile([P, 2 * P], BF16)
            nc.gpsimd.memset(m2[:, 0:P], 0.0)
            make_upper_triangular(nc, m2[:, P : 2 * P], val=1.0, diag=True)

            wo_sb = consts.tile([P, KCH, D_MODEL], BF16)
            hres_sb = consts.tile([P, n_it, D_MODEL], F32)
            if apply_gb:
                g_sb = consts.tile([P, D_MODEL], F32)
                b_sb = consts.tile([P, D_MODEL], F32)

            for _rep in range(repeat):
                # ---- A2A buffers (a single exchange covering both batches;
                # destination core c receives, for each batch, its token
                # chunk of this core's two heads at columns b*cs..b*cs+cs) ----
                av_in = dram.tile([n_cores, P, B * cs], F8, name="av_in")
                av_out = dram.tile([n_cores, P, B * cs], F8, name="av_out")

                # q/k/v kept transposed: [128 rows = 2 heads x 64 dims, tokens]
                qT_sb = qkvp.tile([P, B, t], BF16)
                kT_sb = qkvp.tile([P, B, t], BF16)
                vT_sb = qkvp.tile([P, B, t], BF16)
                # vext[b]: [128, nt, 2, 65]; col 64 of each head stays 1.0
                # (ones-column sumexp trick); filled by PE transposes of vT
                vext = []
                for b in range(B):
                    v = qkvp.tile([P, nt, nh_loc, D_HEAD + 1], BF16, name=f"vext_{b}")
                    if skip_proj:
                        nc.vector.memset(v, 0.01)
                    nc.vector.memset(v[:, :, :, D_HEAD : D_HEAD + 1], 1.0)
                    vext.append(v)

                nsub = D_MODEL // 512  # bn_stats subgroups
                po_parity = 0
                deferred = []

                # Pipeline: proj(b0) first; then attention(b) with batch
                # b+1's projection groups interleaved after each i-block.
                # Attention is Act-bound (exps) while projections are
                # PE-bound, so the PE fills its idle slack with projection
                # matmuls and the serial projection phase between batches
                # disappears. PSUM is conflict-free: projections use the
                # pproj ring, attention uses psc/pav.
                def load_hT(b):
                    # hpool bufs=1: this DMA waits (WAR) until the previous
                    # batch's projections finish reading the slot
                    hT_b = hpool.tile([P, KCH, t], BF16, name="hT_b")
                    for k in range(KCH):
                        nc.sync.dma_start(out=hT_b[:, k, :], in_=hT_d[b, k])
                    return hT_b

                def emit_proj_group(b, hT_b, nb):
                    # ---- Q/K/V projections, all weight-stationary ----
                    for mt in range(3):  # 0 -> q, 1 -> k, 2 -> v
                        ps = pproj.tile([P, 512], F32, tag="proj", name="ps_qkv")
                        for k in range(KCH):
                            nc.tensor.matmul(
                                ps,
                                lhsT=wqkv_sb[:, k, mt * P : (mt + 1) * P],
                                rhs=hT_b[:, k, nb * 512 : (nb + 1) * 512],
                                start=(k == 0),
                                stop=(k == KCH - 1),
                            )
                        dst = (qT_sb, kT_sb, vT_sb)[mt][:, b, nb * 512 : (nb + 1) * 512]
                        nc.vector.tensor_copy(dst, ps)
                    # transpose this group's vT into token-major vext:
                    # 4 [128,128] PE transposes into one PSUM tile, then
                    # one strided copy into vext (both heads at once)
                    pst = pproj.tile([P, 4, P], BF16, tag="proj", name="pst")
                    for i in range(4):
                        jt = 4 * nb + i
                        nc.tensor.transpose(
                            pst[:, i, :],
                            vT_sb[:, b, jt * P : (jt + 1) * P],
                            ident,
                        )
                    dst = vext[b][:, 4 * nb : 4 * nb + 4, :, 0:D_HEAD]
                    src = bass.AP(
                        tensor=pst.tensor,
                        offset=pst.offset,
                        ap=[pst.ap[0], pst.ap[1], [D_HEAD, nh_loc], [1, D_HEAD]],
                    )
                    nc.vector.tensor_copy(dst, src)

                def emit_attn_ib(b, ib):
                    # ---- attention ----
                    # scores kept transposed: s[j, i] for j-tile (128 keys)
                    # x i-block (512 queries); softmax over j via the ones
                    # column in V.
                    if True:
                        if skip_attn:
                            avt = avsb.tile([D_HEAD, nh_loc, 512], F8, tag="avt", name="avt")
                            nc.vector.memset(avt, 0.01)
                            for c in range(IB // cs):
                                nc.sync.dma_start(
                                    out=bass.AP(
                                        tensor=av_in.tensor,
                                        offset=av_in.offset
                                        + (ib * (IB // cs) + c) * P * B * cs
                                        + b * cs,
                                        ap=[[B * cs, D_HEAD], [D_HEAD * B * cs, nh_loc], [1, cs]],
                                    ),
                                    in_=bass.AP(
                                        tensor=avt.tensor,
                                        offset=avt.offset + c * cs,
                                        ap=[avt.ap[0], [IB, nh_loc], [1, cs]],
                                    ),
                                )
                            return
                        njt = 4 * ib + 4  # causal: j-tiles 0..4ib+3
                        avps = [
                            pav.tile([D_HEAD + 1, 512], F32, tag="av", name=f"avps{h}")
                            for h in range(nh_loc)
                        ]
                        njp = njt // 2

                        def pair_off(jp):
                            # causal trim: both tiles of a pair compute query
                            # columns [o0, 512); the odd tile's extra 128
                            # columns are invalid and masked via m2.
                            jt0, jt1 = 2 * jp, 2 * jp + 1
                            o0 = max(0, jt0 * P - ib * IB)
                            o1 = max(0, jt1 * P - ib * IB)
                            return jt0, jt1, o0, o1, jt1 * P - ib * IB >= 0

                        def emit_scores(jp, h):
                            jt0, jt1, o0, _, _ = pair_off(jp)
                            base = h * D_HEAD
                            scp = psc.tile([P, 2, 512], F32, tag="sc", name="scp")
                            for jj, jt in ((0, jt0), (1, jt1)):
                                nc.tensor.matmul(
                                    scp[:, jj, o0:512],
                                    lhsT=kT_sb[base : base + D_HEAD, b, jt * P : (jt + 1) * P],
                                    rhs=qT_sb[base : base + D_HEAD, b, ib * IB + o0 : (ib + 1) * IB],
                                    start=True,
                                    stop=True,
                                )
                            return scp

                        # software pipeline: the next pair's score matmuls are
                        # emitted BEFORE this pair's AV matmuls so the PE feeds
                        # the (pacing) Scalar engine as early as possible
                        scp_cur = [emit_scores(0, h) for h in range(nh_loc)]
                        for jp in range(njp):
                            jt0, jt1, o0, o1, diag = pair_off(jp)
                            expts = []
                            for h in range(nh_loc):
                                expt = expp.tile([P, 2, 512], BF16, tag="exp", name="expt")
                                nc.scalar.activation(
                                    expt[:, :, o0:512],
                                    scp_cur[h][:, :, o0:512],
                                    mybir.ActivationFunctionType.Exp,
                                    bias=expb_sb,
                                )
                                expts.append(expt)
                            if jp + 1 < njp:
                                scp_cur = [emit_scores(jp + 1, h) for h in range(nh_loc)]
                            for h in range(nh_loc):
                                expt = expts[h]
                                if diag:
                                    nc.vector.tensor_mul(
                                        expt[:, 0, o0 : o0 + P], expt[:, 0, o0 : o0 + P], m1
                                    )
                                    nc.vector.tensor_mul(
                                        expt[:, 1, o0 : o0 + 2 * P],
                                        expt[:, 1, o0 : o0 + 2 * P],
                                        m2,
                                    )
                                for jj, jt, oj in ((0, jt0, o0), (1, jt1, o1)):
                                    nc.tensor.matmul(
                                        avps[h][:, oj:512],
                                        lhsT=vext[b][:, jt, h, :],
                                        rhs=expt[:, jj, oj:512],
                                        start=(jt == 0),
                                        stop=(jt == njt - 1),
                                    )
                        # normalize by sumexp (row 64) and ship to the A2A
                        # buffer; both heads batched through one reciprocal /
                        # broadcast / DMA.
                        # sumexp rows: PSUM@p64 -> SBUF@p0 copies (exact),
                        # then reciprocal from SBUF@p0 (approx_fast can't
                        # read shifted PSUM), then broadcast from p0.
                        srow = work.tile([1, nh_loc, 512], F32, tag="srow", name="srow")
                        for h in range(nh_loc):
                            nc.vector.tensor_copy(
                                srow[:, h, :], avps[h][D_HEAD : D_HEAD + 1, :]
                            )
                        rt = work.tile([1, nh_loc, 512], F32, tag="rt", name="rt")
                        nc.vector.reciprocal_approx_fast(out=rt, in_=srow)
                        rb = work.tile([D_HEAD, nh_loc, 512], F32, tag="rb", name="rb")
                        nc.gpsimd.partition_broadcast(rb, rt)
                        avt = avsb.tile([D_HEAD, nh_loc, 512], F8, tag="avt", name="avt")
                        for h in range(nh_loc):
                            nc.vector.tensor_mul(
                                avt[:, h, :], avps[h][0:D_HEAD, :], rb[:, h, :]
                            )
                        # ship to the A2A buffer, one DMA per token chunk
                        # (both heads per DMA):
                        # av_in[ib*IB//cs + c, h*64 + d, b*cs + o] <- avt[d, h, c*cs + o]
                        nchk = IB // cs
                        assert nchk * cs == IB
                        for c in range(nchk):
                            out_ap = bass.AP(
                                tensor=av_in.tensor,
                                offset=av_in.offset
                                + (ib * nchk + c) * P * B * cs
                                + b * cs,
                                ap=[[B * cs, D_HEAD], [D_HEAD * B * cs, nh_loc], [1, cs]],
                            )
                            in_ap = bass.AP(
                                tensor=avt.tensor,
                                offset=avt.offset + c * cs,
                                ap=[avt.ap[0], [IB, nh_loc], [1, cs]],
                            )
                            nc.sync.dma_start(out=out_ap, in_=in_ap)

                # ---- driver: per-batch pipeline (hT DMA -> projections
                # -> attention); batch b+1's hT DMA overlaps batch b's
                # attention via the hpool WAR ----
                if skip_proj and _rep == 0:
                    nc.vector.memset(qT_sb, 0.01)
                    nc.vector.memset(kT_sb, 0.01)
                for b in range(B):
                    if not skip_proj:
                        hT_b = load_hT(b)
                        for nb in range(t // 512):
                            emit_proj_group(b, hT_b, nb)
                    for ib in range(n_ib):
                        emit_attn_ib(b, ib)

                # ---- single AllToAll covering both batches ----
                if no_collective:
                    for k in range(n_cores):
                        nc.sync.dma_start(out=av_out[k], in_=av_in[k])
                else:
                    nc.gpsimd.collective_compute(
                        "AllToAll",
                        mybir.AluOpType.bypass,
                        replica_groups=[list(range(n_cores))],
                        ins=[av_in.opt()],
                        outs=[av_out.opt()],
                    )

                # ---- output projection + residual + LayerNorm for this
                # core's token rows (both batches)
                if _rep == 0:
                    # late-phase constants (emitted here so the DMA queues
                    # serve hT and the qkv weights first at kernel start)
                    nc.sync.dma_start(out=wo_sb, in_=_perm(wo_d, (1, 0, 2)))
                    nc.sync.dma_start(out=hres_sb, in_=_perm(hres_d, (1, 0, 2)))
                    if apply_gb:
                        nc.sync.dma_start(
                            out=g_sb,
                            in_=bass.AP(tensor=g_d.tensor, offset=g_d.offset, ap=[[0, P], *g_d.ap]),
                        )
                        nc.sync.dma_start(
                            out=b_sb,
                            in_=bass.AP(tensor=b_d.tensor, offset=b_d.offset, ap=[[0, P], *b_d.ap]),
                        )

                avg_sb = qkvp.tile([P, n_cores, B * cs], F8, tag="avg", bufs=1, name="avg_sb")
                nc.sync.dma_start(
                    out=avg_sb,
                    in_=bass.AP(
                        tensor=av_out.tensor,
                        offset=av_out.offset,
                        ap=[av_out.ap[1], av_out.ap[0], av_out.ap[2]],
                    ),
                )

                if skip_outproj and _rep == 0:
                    xz = work.tile([P, D_MODEL], F32, tag="xn", name="xz")
                    nc.vector.memset(xz, 0.0)
                    for it in range(n_it):
                        nc.sync.dma_start(out=out_d[it], in_=xz)
                for it in range(0 if skip_outproj else n_it):
                    bb_, i2 = it // tiles_pb, it % tiles_pb
                    # alternate PSUM pools so tile it+1's matmuls pipeline
                    # with tile it's LayerNorm (attention is fully done here)
                    popool = pproj if po_parity == 0 else pav
                    potag = "proj" if po_parity == 0 else "av"
                    po_parity ^= 1
                    pos = [
                        popool.tile([P, 512], F32, tag=potag, name=f"po{nh}")
                        for nh in range(2)
                    ]
                    for nh in range(2):
                        for k in range(n_cores):
                            nc.tensor.matmul(
                                pos[nh],
                                lhsT=avg_sb[:, k, bb_ * cs + i2 * P : bb_ * cs + (i2 + 1) * P],
                                rhs=wo_sb[:, k, nh * 512 : (nh + 1) * 512],
                                start=(k == 0),
                                stop=(k == n_cores - 1),
                            )
                    x = defer_pool.tile([P, D_MODEL], F32, tag="x", name="x")
                    for nh in range(2):
                        nc.vector.tensor_add(
                            x[:, nh * 512 : (nh + 1) * 512],
                            pos[nh],
                            hres_sb[:, it, nh * 512 : (nh + 1) * 512],
                        )
                    stats = work.tile([P, nsub, 6], F32, tag="stats", name="stats")
                    for s in range(nsub):
                        nc.vector.bn_stats(stats[:, s, :], x[:, s * 512 : (s + 1) * 512])
                    mv = defer_pool.tile([P, 2], F32, tag="mv", name="mv")
                    nc.vector.bn_aggr(mv, stats)
                    # the sqrt + scale are deferred to the kernel tail so
                    # the sqrt ACT-table load doesn't thrash with the
                    # attention exps (different table sets)
                    deferred.append((it, x, mv))

                # ---- deferred LayerNorm tails (one sqrt table switch) ----
                for it, x, mv in deferred:
                    std = work.tile([P, 1], F32, tag="std", name="std")
                    nc.scalar.activation(
                        std, mv[:, 1:2], mybir.ActivationFunctionType.Sqrt, bias=eps_sb
                    )
                    rstd = work.tile([P, 1], F32, tag="rstd", name="rstd")
                    nc.vector.reciprocal(rstd, std)
                    xn = work.tile([P, D_MODEL], F32, tag="xn", name="xn")
                    nc.vector.tensor_scalar(
                        out=xn,
                        in0=x,
                        scalar1=mv[:, 0:1],
                        scalar2=rstd,
                        op0=mybir.AluOpType.subtract,
                        op1=mybir.AluOpType.mult,
                    )
                    if apply_gb:
                        nc.vector.tensor_mul(xn, xn, g_sb)
                        nc.vector.tensor_add(xn, xn, b_sb)
                    nc.sync.dma_start(out=out_d[it], in_=xn)
                deferred.clear()

    nc.compile()
    return nc


def make_in_maps(h, Wq, Wkv, Wo, ln_g, ln_b, t=T_FULL, n_cores=N_CORES):
    """Builds the per-core input maps (host-side sharding/layout prep)."""
    bf = ml_dtypes.bfloat16
    nh_loc = N_HEAD // n_cores
    cs = t // n_cores
    n_it = B * cs // P

    # hT: [B, KCH, P, t] = h transposed per batch, bf16 (shared by all cores)
    hT = np.ascontiguousarray(h.transpose(1, 2, 0)).reshape(B, KCH, P, t).astype(bf)
    # residual in batch-major token order
    h_bmaj = np.ascontiguousarray(h.transpose(1, 0, 2)).reshape(B * t, D_MODEL)
    g = np.ascontiguousarray(ln_g, dtype=np.float32)
    bvec = np.ascontiguousarray(ln_b, dtype=np.float32)
    wo = np.ascontiguousarray(Wo).reshape(KCH, P, D_MODEL).astype(bf)

    in_maps = []
    for c in range(n_cores):
        heads = [c * nh_loc + i for i in range(nh_loc)]
        # Wq columns for my heads, with the 1/sqrt(d) scale folded in
        wq_cols = [Wq[:, hd * D_HEAD : (hd + 1) * D_HEAD] * SCALE for hd in heads]
        # Wkv: head hd occupies cols [hd*128, hd*128+64) = K, [+64, +128) = V
        wk_cols = [Wkv[:, hd * 2 * D_HEAD : hd * 2 * D_HEAD + D_HEAD] for hd in heads]
        wv_cols = [Wkv[:, hd * 2 * D_HEAD + D_HEAD : (hd + 1) * 2 * D_HEAD] for hd in heads]
        wqkv = np.concatenate(wq_cols + wk_cols + wv_cols, axis=1)  # [1024, 384]
        hres = np.concatenate(
            [h_bmaj[b * t + c * cs : b * t + (c + 1) * cs] for b in range(B)]
        ).reshape(n_it, P, D_MODEL)
        in_maps.append(
            {
                "hT": hT,
                "wqkv": np.ascontiguousarray(wqkv.reshape(KCH, P, 3 * nh_loc * D_HEAD)).astype(bf),
                "wo": wo,
                "hres": np.ascontiguousarray(hres, dtype=np.float32),
                "lng": g,
                "lnb": bvec,
            }
        )
    return in_maps


def assemble_output(results, t=T_FULL, n_cores=N_CORES):
    cs = t // n_cores
    chunks = [results[c]["out"].reshape(B, cs, D_MODEL) for c in range(n_cores)]
    # chunks[c][b] = batch-b tokens [c*cs, (c+1)*cs)
    full = np.concatenate(chunks, axis=1)  # [B, t, D]
    return np.ascontiguousarray(full.transpose(1, 0, 2))


def _numpy_fallback(h, attn_mask, Wq, Wkv, Wo, ln_g, ln_b):
    """Exact reference computation (only used if the mask is not causal)."""
    t, b, _ = h.shape
    hf = h.reshape(t * b, D_MODEL)
    q = (hf @ Wq).reshape(t, b, N_HEAD, D_HEAD)
    kv = (hf @ Wkv).reshape(t, b, N_HEAD, 2 * D_HEAD)
    k, v = kv[..., :D_HEAD], kv[..., D_HEAD:]
    s = np.einsum("ibnd,jbnd->ijbn", q, k) * SCALE
    s = np.where(attn_mask[:, :, :, None], -np.inf, s)
    s = s - s.max(axis=1, keepdims=True)
    p = np.exp(s)
    p = p / p.sum(axis=1, keepdims=True)
    av = np.einsum("ijbn,jbnd->ibnd", p, v).reshape(t, b, N_HEAD * D_HEAD)
    ao = av @ Wo
    x = h + ao
    mu = x.mean(axis=-1, keepdims=True)
    var = ((x - mu) ** 2).mean(axis=-1, keepdims=True)
    return ((x - mu) / np.sqrt(var + LN_EPS) * ln_g + ln_b).astype(np.float32)


_PROGRAM_CACHE = {}


def kernel(h, attn_mask, Wq, Wkv, Wo, ln_g, ln_b):
    global LAST_RESULT
    h = np.asarray(h, dtype=np.float32)
    attn_mask = np.asarray(attn_mask)
    Wq = np.asarray(Wq, dtype=np.float32)
    Wkv = np.asarray(Wkv, dtype=np.float32)
    Wo = np.asarray(Wo, dtype=np.float32)
    ln_g = np.asarray(ln_g, dtype=np.float32)
    ln_b = np.asarray(ln_b, dtype=np.float32)

    t = h.shape[0]
    causal = np.triu(np.ones((t, t), dtype=bool), k=1)
    if not np.array_equal(attn_mask, np.broadcast_to(causal[:, :, None], attn_mask.shape)):
        return _numpy_fallback(h, attn_mask, Wq, Wkv, Wo, ln_g, ln_b)

    apply_gb = not (np.all(ln_g == 1.0) and np.all(ln_b == 0.0))
    key = (t, apply_gb)
    if key not in _PROGRAM_CACHE:
        _PROGRAM_CACHE[key] = build_program(t=t, apply_gb=apply_gb)
    nc = _PROGRAM_CACHE[key]

    in_maps = make_in_maps(h, Wq, Wkv, Wo, ln_g, ln_b, t=t)
    res = run_bass_kernel_spmd(
        nc,
        in_maps,
        core_ids=list(range(N_CORES)),
        trace=bool(int(os.environ.get("KERNEL_TRACE", "0"))),
    )
    LAST_RESULT = res
    return assemble_output(res.results, t=t)


if __name__ == "__main__":
    # quick smoke: random small check vs numpy fallback path is not possible
    # (device required); just build the program.
    build_program()
    print("program built ok")

